# revision 4
# baseline (speedup 1.0000x reference)
"""Cosine multihead attention on 8 Trainium2 NeuronCores.

Sharding: batch*heads across cores. Core c handles batch b = c // 4 and the
4 heads [4*(c%4), 4*(c%4)+4). Each core computes its heads' q/k/v projections
(tensor-parallel slices of in_proj), full attention for its (B,H) slice, and a
partial out-projection (rank-256 contribution). The host sums the 4 partials
per batch and adds out_proj_bias.

Device layout notes:
- q,k are projected directly in transposed orientation (head_dim on
  partitions, seq on free) so QK^T needs no on-chip transpose; v is projected
  in natural orientation so it is directly the PV stationary operand.
- QK^T runs 2 heads concurrently via PE row tiling (K=64 at partition bases
  0 and 64).
- Softmax denominators come free from a ones-column appended to v (M=65 PV).
- All bf16 matmuls with fp32 PSUM accumulation; softmax/normalization math in
  fp32.
"""

import sys

if "/opt/trn_rl_repo" not in sys.path:
    sys.path.insert(0, "/opt/trn_rl_repo")

import numpy as np
import ml_dtypes

import concourse.bass as bass
import concourse.tile as tile
from concourse import bacc, mybir
from concourse.bass_utils import run_bass_kernel_spmd

S, B, E, H = 2048, 2, 1024, 16
HD = E // H            # 64
HPC = 4                # heads per core
NCORES = 8
TAU_MIN = 0.01

BF16 = ml_dtypes.bfloat16
DT_BF = mybir.dt.bfloat16
DT_F32 = mybir.dt.float32

KC_E = E // 128        # 8 contraction chunks for projections
MQ = S // 128          # 16 seq chunks of 128
NPAIR = HPC // 2       # 2 head pairs per core


def build_program():
    """Build the SPMD per-core Bass program. Same program on all 8 cores;
    all per-core differences live in the input data."""
    nc = bacc.Bacc(None)

    xq = nc.dram_tensor("xq_t", [E, S], DT_BF, kind="ExternalInput")
    xk = nc.dram_tensor("xk_t", [E, S], DT_BF, kind="ExternalInput")
    xv = nc.dram_tensor("xv_t", [E, S], DT_BF, kind="ExternalInput")
    wq = nc.dram_tensor("wq_t", [E, 256], DT_BF, kind="ExternalInput")
    wk = nc.dram_tensor("wk_t", [E, 256], DT_BF, kind="ExternalInput")
    wv = nc.dram_tensor("wv_t", [E, 256], DT_BF, kind="ExternalInput")
    bq = nc.dram_tensor("b_q", [1, 256], DT_BF, kind="ExternalInput")
    bk = nc.dram_tensor("b_k", [1, 256], DT_BF, kind="ExternalInput")
    bv = nc.dram_tensor("b_v", [1, 256], DT_BF, kind="ExternalInput")
    wo = nc.dram_tensor("wo_t", [256, E], DT_BF, kind="ExternalInput")
    selk_in = nc.dram_tensor("selk", [2, 256], DT_F32, kind="ExternalInput")
    selq_in = nc.dram_tensor("selq", [2, 128], DT_F32, kind="ExternalInput")
    outp = nc.dram_tensor("out_p", [S, E], DT_F32, kind="ExternalOutput")

    with tile.TileContext(nc) as tc:
        with (
            tc.tile_pool(name="consts", bufs=1) as consts,
            tc.tile_pool(name="xin", bufs=1) as xin,
            tc.tile_pool(name="wts", bufs=1) as wts,
            tc.tile_pool(name="qk", bufs=1) as qkpool,
            tc.tile_pool(name="vsb", bufs=1) as vpool,
            tc.tile_pool(name="work", bufs=2) as work,
            tc.tile_pool(name="outs", bufs=2) as outs,
            tc.tile_pool(name="ps_mm", bufs=2, space="PSUM") as ps_mm,
            tc.tile_pool(name="ps_acc", bufs=3, space="PSUM") as ps_acc,
            tc.tile_pool(name="ps_zb", bufs=1, space="PSUM") as ps_zb,
        ):
            # ---- constants -------------------------------------------------
            ones_row = consts.tile([1, 512], DT_BF, tag="ones_row")
            nc.vector.memset(ones_row, 1.0)
            ones_hi = consts.tile([128, 64], DT_F32, tag="ones_hi")
            nc.vector.memset(ones_hi, 1.0)
            selq = consts.tile([2, 128], DT_F32, tag="selq")
            nc.sync.dma_start(out=selq, in_=selq_in[:, :])
            hsel = consts.tile([128, 2], DT_BF, tag="hsel")
            nc.vector.memset(hsel, 0.0)
            nc.vector.memset(hsel[0:64, 0:1], 1.0)
            nc.vector.memset(hsel[64:128, 1:2], 1.0)
            selk_sb = consts.tile([2, 256], DT_F32, tag="selk")
            nc.sync.dma_start(out=selk_sb, in_=selk_in[:, :])

            bq_sb = consts.tile([1, 256], DT_BF, tag="bq")
            bk_sb = consts.tile([1, 256], DT_BF, tag="bk")
            bv_sb = consts.tile([1, 256], DT_BF, tag="bv")
            nc.sync.dma_start(out=bq_sb, in_=bq[:, :])
            nc.sync.dma_start(out=bk_sb, in_=bk[:, :])
            nc.sync.dma_start(out=bv_sb, in_=bv[:, :])

            # ---- weights ---------------------------------------------------
            wq_sb = wts.tile([128, KC_E, 256], DT_BF, tag="wq")
            wk_sb = wts.tile([128, KC_E, 256], DT_BF, tag="wk")
            wv_sb = wts.tile([128, KC_E, 256], DT_BF, tag="wv")
            for c in range(KC_E):
                nc.sync.dma_start(out=wq_sb[:, c, :], in_=wq[c * 128:(c + 1) * 128, :])
                nc.sync.dma_start(out=wk_sb[:, c, :], in_=wk[c * 128:(c + 1) * 128, :])
                nc.sync.dma_start(out=wv_sb[:, c, :], in_=wv[c * 128:(c + 1) * 128, :])
            wo_sb = wts.tile([128, 2, E], DT_BF, tag="wo")
            for c in range(2):
                nc.sync.dma_start(out=wo_sb[:, c, :], in_=wo[c * 128:(c + 1) * 128, :])

            # ---- activations (kept fully resident) -------------------------
            xq_sb = xin.tile([128, KC_E, S], DT_BF, tag="xq")
            xk_sb = xin.tile([128, KC_E, S], DT_BF, tag="xk")
            xv_sb = xin.tile([128, KC_E, S], DT_BF, tag="xv")
            for c in range(KC_E):
                nc.sync.dma_start(out=xq_sb[:, c, :], in_=xq[c * 128:(c + 1) * 128, :])
                nc.sync.dma_start(out=xk_sb[:, c, :], in_=xk[c * 128:(c + 1) * 128, :])
                nc.sync.dma_start(out=xv_sb[:, c, :], in_=xv[c * 128:(c + 1) * 128, :])

            # ---- v projection (natural orientation) ------------------------
            # v_sb[:, m, h, 0:64] = v rows m*128..+128 for head h;
            # column 64 is ones (softmax denominator trick).
            v_sb = vpool.tile([128, MQ, HPC, HD + 1], DT_BF, tag="v")
            nc.vector.memset(v_sb[:, :, :, HD:HD + 1], 1.0)
            for m in range(MQ):
                vp = ps_acc.tile([128, 256], DT_F32, tag="oacc")
                for c in range(KC_E):
                    nc.tensor.matmul(
                        vp,
                        lhsT=xv_sb[:, c, m * 128:(m + 1) * 128],
                        rhs=wv_sb[:, c, :],
                        start=(c == 0),
                        stop=False,
                    )
                nc.tensor.matmul(
                    vp,
                    lhsT=ones_row[0:1, 0:128],
                    rhs=bv_sb[0:1, :],
                    start=False,
                    stop=True,
                )
                nc.vector.tensor_copy(
                    out=v_sb[:, m, :, 0:HD],
                    in_=vp.rearrange("p (h d) -> p h d", h=HPC),
                )

            # ---- q/k projections (transposed) + cosine normalization -------
            # pair tiles: rows 0-63 head (2*pair), rows 64-127 head (2*pair+1)
            qt = [qkpool.tile([128, S], DT_BF, tag=f"qt{p}", name=f"qt{p}") for p in range(NPAIR)]
            kt = [qkpool.tile([128, S], DT_BF, tag=f"kt{p}", name=f"kt{p}") for p in range(NPAIR)]

            for t_sb, w_sb, b_sb, sel, x_sb in (
                (qt, wq_sb, bq_sb, selq, xq_sb),
                (kt, wk_sb, bk_sb, None, xk_sb),
            ):
                for mc in range(NPAIR):
                    dst = t_sb[mc]
                    for qb2 in range(2):  # 1024-wide units
                        pp = ps_mm.tile([128, 1024], DT_F32, tag="sc")
                        for c in range(KC_E):
                            for n2 in range(2):
                                nc.tensor.matmul(
                                    pp[:, n2 * 512:(n2 + 1) * 512],
                                    lhsT=w_sb[:, c, mc * 128:(mc + 1) * 128],
                                    rhs=x_sb[:, c, qb2 * 1024 + n2 * 512:
                                             qb2 * 1024 + (n2 + 1) * 512],
                                    start=(c == 0),
                                    stop=False,
                                )
                        for n2 in range(2):
                            nc.tensor.matmul(
                                pp[:, n2 * 512:(n2 + 1) * 512],
                                lhsT=b_sb[0:1, mc * 128:(mc + 1) * 128],
                                rhs=ones_row[0:1, 0:512],
                                start=False,
                                stop=True,
                            )
                        sl1024 = slice(qb2 * 1024, (qb2 + 1) * 1024)
                        # raw (biased) values, bf16
                        nc.vector.tensor_copy(out=dst[:, sl1024], in_=pp)
                        sqt = work.tile([128, 1024], DT_BF, tag="sq")
                        nc.vector.tensor_mul(sqt, dst[:, sl1024], dst[:, sl1024])
                        for n2 in range(2):
                            sl512 = slice(qb2 * 1024 + n2 * 512,
                                          qb2 * 1024 + (n2 + 1) * 512)
                            ss = ps_acc.tile([2, 512], DT_F32, tag="oacc")
                            nc.tensor.matmul(
                                ss,
                                lhsT=hsel,
                                rhs=sqt[:, n2 * 512:(n2 + 1) * 512],
                                start=True,
                                stop=True,
                            )
                            st = work.tile([2, 512], DT_F32, tag="st")
                            nc.scalar.activation(
                                st, ss, mybir.ActivationFunctionType.Sqrt
                            )
                            rn = work.tile([2, 512], DT_F32, tag="rn")
                            nc.vector.reciprocal(rn, st)
                            rb = ps_acc.tile([128, 512], DT_F32, tag="oacc")
                            lhs_sel = (
                                selq if sel is not None
                                else selk_sb[:, mc * 128:(mc + 1) * 128]
                            )
                            nc.tensor.matmul(
                                rb, lhsT=lhs_sel, rhs=rn, start=True, stop=True
                            )
                            # normalize in place (k side also folds 1/tau)
                            nc.vector.tensor_mul(dst[:, sl512], dst[:, sl512], rb)

            # ---- attention per head pair ------------------------------------
            heads_t = [qkpool.tile([128, S], DT_BF, tag=f"ht{p}", name=f"ht{p}") for p in range(NPAIR)]
            for p in range(NPAIR):
                for qb in range(4):  # 512-wide query blocks
                    sl_q = slice(qb * 512, (qb + 1) * 512)
                    o0 = ps_acc.tile([128, 512], DT_F32, tag="oacc")
                    o1 = ps_acc.tile([128, 512], DT_F32, tag="oacc")
                    for kc in range(MQ):
                        sc = ps_mm.tile([128, 1024], DT_F32, tag="sc")
                        nc.tensor.matmul(
                            sc[:, 0:512],
                            lhsT=kt[p][0:64, kc * 128:(kc + 1) * 128],
                            rhs=qt[p][0:64, sl_q],
                            start=True,
                            stop=True,
                        )
                        nc.tensor.matmul(
                            sc[:, 512:1024],
                            lhsT=kt[p][64:128, kc * 128:(kc + 1) * 128],
                            rhs=qt[p][64:128, sl_q],
                            start=True,
                            stop=True,
                        )
                        ex = work.tile([128, 1024], DT_BF, tag="exp")
                        nc.scalar.activation(
                            ex, sc, mybir.ActivationFunctionType.Exp
                        )
                        nc.tensor.matmul(
                            o0[0:65, :],
                            lhsT=v_sb[:, kc, 2 * p, :],
                            rhs=ex[:, 0:512],
                            start=(kc == 0),
                            stop=(kc == MQ - 1),
                        )
                        nc.tensor.matmul(
                            o1[0:65, :],
                            lhsT=v_sb[:, kc, 2 * p + 1, :],
                            rhs=ex[:, 512:1024],
                            start=(kc == 0),
                            stop=(kc == MQ - 1),
                        )
                    for hl, o in ((0, o0), (1, o1)):
                        zi = work.tile([128, 512], DT_F32, tag="zi")
                        nc.vector.reciprocal(zi[64:65, :], o[64:65, :])
                        zb = ps_zb.tile([64, 512], DT_F32, tag="zb")
                        nc.tensor.matmul(
                            zb,
                            lhsT=ones_hi[64:65, 0:64],
                            rhs=zi[64:65, :],
                            start=True,
                            stop=True,
                        )
                        ot = work.tile([64, 512], DT_F32, tag="ot")
                        nc.vector.tensor_copy(ot, o[0:64, :])
                        if hl == 0:
                            nc.vector.tensor_mul(heads_t[p][0:64, sl_q], ot, zb)
                        else:
                            t2 = work.tile([64, 512], DT_BF, tag="t2")
                            nc.vector.tensor_mul(t2, ot, zb)
                            nc.sync.dma_start(
                                out=heads_t[p][64:128, sl_q], in_=t2
                            )

            # ---- partial out-projection ------------------------------------
            for m in range(MQ):
                op = ps_mm.tile([128, 1024], DT_F32, tag="sc")
                for c in range(2):
                    for n2 in range(2):
                        nc.tensor.matmul(
                            op[:, n2 * 512:(n2 + 1) * 512],
                            lhsT=heads_t[c][:, m * 128:(m + 1) * 128],
                            rhs=wo_sb[:, c, n2 * 512:(n2 + 1) * 512],
                            start=(c == 0),
                            stop=(c == 1),
                        )
                ob = outs.tile([128, 1024], DT_F32, tag="ob")
                nc.vector.tensor_copy(ob, op)
                nc.sync.dma_start(out=outp[m * 128:(m + 1) * 128, :], in_=ob)

    nc.compile()
    return nc


_CACHE = {}


def _get_program():
    if "nc" not in _CACHE:
        _CACHE["nc"] = build_program()
    return _CACHE["nc"]


def make_in_maps(query, key, value, in_proj_weight, in_proj_bias,
                 out_proj_weight, out_proj_bias, tau):
    query = np.asarray(query, np.float32)
    key = np.asarray(key, np.float32)
    value = np.asarray(value, np.float32)
    W = np.asarray(in_proj_weight, np.float32)
    bias = np.asarray(in_proj_bias, np.float32)
    Wo = np.asarray(out_proj_weight, np.float32)
    inv_tau = 1.0 / np.maximum(np.asarray(tau, np.float32).reshape(H), TAU_MIN)

    # Transposed activations per batch: (E, S) bf16
    xT = {}
    for b in range(B):
        xT["q", b] = np.ascontiguousarray(query[:, b, :].T).astype(BF16)
        xT["k", b] = np.ascontiguousarray(key[:, b, :].T).astype(BF16)
        xT["v", b] = np.ascontiguousarray(value[:, b, :].T).astype(BF16)

    selq_host = np.zeros((2, 128), np.float32)
    selq_host[0, 0:64] = 1.0
    selq_host[1, 64:128] = 1.0
    in_maps = []
    for c in range(NCORES):
        b = c // 4
        h0 = HPC * (c % 4)
        rows = slice(h0 * HD, (h0 + HPC) * HD)
        rows_k = slice(E + h0 * HD, E + (h0 + HPC) * HD)
        rows_v = slice(2 * E + h0 * HD, 2 * E + (h0 + HPC) * HD)
        # per-pair selector with 1/tau folded in for the k side
        selk = np.zeros((2, 256), np.float32)
        for mc in range(NPAIR):
            selk[0, mc * 128:mc * 128 + 64] = inv_tau[h0 + 2 * mc]
            selk[1, mc * 128 + 64:(mc + 1) * 128] = inv_tau[h0 + 2 * mc + 1]
        in_maps.append({
            "xq_t": xT["q", b],
            "xk_t": xT["k", b],
            "xv_t": xT["v", b],
            "wq_t": np.ascontiguousarray(W[rows, :].T).astype(BF16),
            "wk_t": np.ascontiguousarray(W[rows_k, :].T).astype(BF16),
            "wv_t": np.ascontiguousarray(W[rows_v, :].T).astype(BF16),
            "b_q": bias[rows].reshape(1, 256).astype(BF16),
            "b_k": bias[rows_k].reshape(1, 256).astype(BF16),
            "b_v": bias[rows_v].reshape(1, 256).astype(BF16),
            "wo_t": np.ascontiguousarray(Wo[:, rows].T).astype(BF16),
            "selk": selk,
            "selq": selq_host,
        })
    return in_maps


def assemble_out(results, out_proj_bias):
    bo = np.asarray(out_proj_bias, np.float32)
    out = np.zeros((S, B, E), np.float32)
    for c in range(NCORES):
        out[:, c // 4, :] += results[c]["out_p"]
    out += bo[None, None, :]
    return out


def kernel(query, key, value, in_proj_weight, in_proj_bias,
           out_proj_weight, out_proj_bias, tau):
    nc = _get_program()
    in_maps = make_in_maps(query, key, value, in_proj_weight, in_proj_bias,
                           out_proj_weight, out_proj_bias, tau)
    res = run_bass_kernel_spmd(nc, in_maps, core_ids=list(range(NCORES)))
    return assemble_out(res.results, out_proj_bias)


if __name__ == "__main__":
    import reference

    inputs = {k: np.asarray(v) for k, v in reference.setup_inputs().items()}
    out = kernel(**inputs)
    print("out shape", out.shape, out.dtype)


# revision 6
# speedup vs baseline: 1.2902x; 1.2902x over previous
"""Cosine multihead attention on 8 Trainium2 NeuronCores.

Sharding: batch*heads across cores. Core c handles batch b = c // 4 and the
4 heads [4*(c%4), 4*(c%4)+4). Each core computes its heads' q/k/v projections
(tensor-parallel slices of in_proj), full attention for its (B,H) slice, and a
partial out-projection (rank-256 contribution). The host sums the 4 partials
per batch and adds out_proj_bias.

Device layout notes:
- q,k are projected directly in transposed orientation (head_dim on
  partitions, seq on free) so QK^T needs no on-chip transpose; v is projected
  in natural orientation so it is directly the PV stationary operand.
- QK^T runs 2 heads concurrently via PE row tiling (K=64 at partition bases
  0 and 64).
- Softmax denominators come free from a ones-column appended to v (M=65 PV).
- All bf16 matmuls with fp32 PSUM accumulation; softmax/normalization math in
  fp32.
"""

import sys

if "/opt/trn_rl_repo" not in sys.path:
    sys.path.insert(0, "/opt/trn_rl_repo")

import numpy as np
import ml_dtypes

import concourse.bass as bass
import concourse.tile as tile
from concourse import bacc, mybir
from concourse.bass_utils import run_bass_kernel_spmd

S, B, E, H = 2048, 2, 1024, 16
HD = E // H            # 64
HPC = 4                # heads per core
NCORES = 8
TAU_MIN = 0.01

BF16 = ml_dtypes.bfloat16
DT_BF = mybir.dt.bfloat16
DT_F32 = mybir.dt.float32

KC_E = E // 128        # 8 contraction chunks for projections
MQ = S // 128          # 16 seq chunks of 128
NPAIR = HPC // 2       # 2 head pairs per core


def build_program():
    """Build the SPMD per-core Bass program. Same program on all 8 cores;
    all per-core differences live in the input data."""
    nc = bacc.Bacc(None)

    xq = nc.dram_tensor("xq_t", [E, S], DT_BF, kind="ExternalInput")
    xk = nc.dram_tensor("xk_t", [E, S], DT_BF, kind="ExternalInput")
    xv = nc.dram_tensor("xv_t", [E, S], DT_BF, kind="ExternalInput")
    wq = nc.dram_tensor("wq_t", [E, 256], DT_BF, kind="ExternalInput")
    wk = nc.dram_tensor("wk_t", [E, 256], DT_BF, kind="ExternalInput")
    wv = nc.dram_tensor("wv_t", [E, 256], DT_BF, kind="ExternalInput")
    bq = nc.dram_tensor("b_q", [1, 256], DT_BF, kind="ExternalInput")
    bk = nc.dram_tensor("b_k", [1, 256], DT_BF, kind="ExternalInput")
    bv = nc.dram_tensor("b_v", [1, 256], DT_BF, kind="ExternalInput")
    wo = nc.dram_tensor("wo_t", [256, E], DT_BF, kind="ExternalInput")
    selk_in = nc.dram_tensor("selk", [2, 256], DT_F32, kind="ExternalInput")
    selq_in = nc.dram_tensor("selq", [2, 128], DT_F32, kind="ExternalInput")
    outp = nc.dram_tensor("out_p", [S, E], DT_F32, kind="ExternalOutput")

    with tile.TileContext(nc) as tc:
        with (
            tc.tile_pool(name="consts", bufs=1) as consts,
            tc.tile_pool(name="xin", bufs=1) as xin,
            tc.tile_pool(name="wts", bufs=1) as wts,
            tc.tile_pool(name="qk", bufs=1) as qkpool,
            tc.tile_pool(name="vsb", bufs=1) as vpool,
            tc.tile_pool(name="work", bufs=2) as work,
            tc.tile_pool(name="outs", bufs=2) as outs,
            tc.tile_pool(name="ps_mm", bufs=2, space="PSUM") as ps_mm,
            tc.tile_pool(name="ps_acc", bufs=3, space="PSUM") as ps_acc,
            tc.tile_pool(name="ps_zb", bufs=1, space="PSUM") as ps_zb,
        ):
            # ---- constants -------------------------------------------------
            ones_row = consts.tile([1, 512], DT_BF, tag="ones_row")
            nc.vector.memset(ones_row, 1.0)
            ones_hi = consts.tile([128, 64], DT_F32, tag="ones_hi")
            nc.vector.memset(ones_hi, 1.0)
            selq = consts.tile([2, 128], DT_F32, tag="selq")
            nc.sync.dma_start(out=selq, in_=selq_in[:, :])
            hsel = consts.tile([128, 2], DT_BF, tag="hsel")
            nc.vector.memset(hsel, 0.0)
            nc.vector.memset(hsel[0:64, 0:1], 1.0)
            nc.vector.memset(hsel[64:128, 1:2], 1.0)
            selk_sb = consts.tile([2, 256], DT_F32, tag="selk")
            nc.sync.dma_start(out=selk_sb, in_=selk_in[:, :])

            bq_sb = consts.tile([1, 256], DT_BF, tag="bq")
            bk_sb = consts.tile([1, 256], DT_BF, tag="bk")
            bv_sb = consts.tile([1, 256], DT_BF, tag="bv")
            nc.sync.dma_start(out=bq_sb, in_=bq[:, :])
            nc.sync.dma_start(out=bk_sb, in_=bk[:, :])
            nc.sync.dma_start(out=bv_sb, in_=bv[:, :])

            # ---- weights ---------------------------------------------------
            wq_sb = wts.tile([128, KC_E, 256], DT_BF, tag="wq")
            wk_sb = wts.tile([128, KC_E, 256], DT_BF, tag="wk")
            wv_sb = wts.tile([128, KC_E, 256], DT_BF, tag="wv")
            for c in range(KC_E):
                nc.sync.dma_start(out=wq_sb[:, c, :], in_=wq[c * 128:(c + 1) * 128, :])
                nc.sync.dma_start(out=wk_sb[:, c, :], in_=wk[c * 128:(c + 1) * 128, :])
                nc.sync.dma_start(out=wv_sb[:, c, :], in_=wv[c * 128:(c + 1) * 128, :])
            wo_sb = wts.tile([128, 2, E], DT_BF, tag="wo")
            for c in range(2):
                nc.sync.dma_start(out=wo_sb[:, c, :], in_=wo[c * 128:(c + 1) * 128, :])

            # ---- activations (kept fully resident) -------------------------
            xq_sb = xin.tile([128, KC_E, S], DT_BF, tag="xq")
            xk_sb = xin.tile([128, KC_E, S], DT_BF, tag="xk")
            xv_sb = xin.tile([128, KC_E, S], DT_BF, tag="xv")
            for x_sb_t, x_dr in ((xq_sb, xq), (xk_sb, xk), (xv_sb, xv)):
                for c in range(KC_E):
                    nc.sync.dma_start(
                        out=x_sb_t[:, c, :], in_=x_dr[c * 128:(c + 1) * 128, :]
                    )

            # ---- q/k projections (transposed) + cosine normalization -------
            # pair tiles: rows 0-63 head (2*pair), rows 64-127 head (2*pair+1)
            qt = [qkpool.tile([128, S], DT_BF, tag=f"qt{p}", name=f"qt{p}") for p in range(NPAIR)]
            kt = [qkpool.tile([128, S], DT_BF, tag=f"kt{p}", name=f"kt{p}") for p in range(NPAIR)]

            for t_sb, w_sb, b_sb, sel, x_sb in (
                (qt, wq_sb, bq_sb, selq, xq_sb),
                (kt, wk_sb, bk_sb, None, xk_sb),
            ):
                for mc in range(NPAIR):
                    dst = t_sb[mc]
                    for qb2 in range(2):  # 1024-wide units
                        pp = ps_mm.tile([128, 1024], DT_F32, tag="sc")
                        for c in range(KC_E):
                            for n2 in range(2):
                                nc.tensor.matmul(
                                    pp[:, n2 * 512:(n2 + 1) * 512],
                                    lhsT=w_sb[:, c, mc * 128:(mc + 1) * 128],
                                    rhs=x_sb[:, c, qb2 * 1024 + n2 * 512:
                                             qb2 * 1024 + (n2 + 1) * 512],
                                    start=(c == 0),
                                    stop=False,
                                )
                        for n2 in range(2):
                            nc.tensor.matmul(
                                pp[:, n2 * 512:(n2 + 1) * 512],
                                lhsT=b_sb[0:1, mc * 128:(mc + 1) * 128],
                                rhs=ones_row[0:1, 0:512],
                                start=False,
                                stop=True,
                            )
                        sl1024 = slice(qb2 * 1024, (qb2 + 1) * 1024)
                        # raw (biased) values, bf16
                        nc.vector.tensor_copy(out=dst[:, sl1024], in_=pp)
                        sqt = work.tile([128, 1024], DT_BF, tag="sq")
                        nc.vector.tensor_mul(sqt, dst[:, sl1024], dst[:, sl1024])
                        for n2 in range(2):
                            sl512 = slice(qb2 * 1024 + n2 * 512,
                                          qb2 * 1024 + (n2 + 1) * 512)
                            ss = ps_acc.tile([2, 512], DT_F32, tag="oacc")
                            nc.tensor.matmul(
                                ss,
                                lhsT=hsel,
                                rhs=sqt[:, n2 * 512:(n2 + 1) * 512],
                                start=True,
                                stop=True,
                            )
                            st = work.tile([2, 512], DT_F32, tag="st")
                            nc.scalar.activation(
                                st, ss, mybir.ActivationFunctionType.Sqrt
                            )
                            rb = ps_acc.tile([128, 512], DT_F32, tag="oacc")
                            lhs_sel = (
                                selq if sel is not None
                                else selk_sb[:, mc * 128:(mc + 1) * 128]
                            )
                            # rb = broadcast of (tau *) ||x|| per head
                            nc.tensor.matmul(
                                rb, lhsT=lhs_sel, rhs=st, start=True, stop=True
                            )
                            rbi = work.tile([128, 512], DT_F32, tag="rbi")
                            nc.vector.reciprocal_approx_fast(out=rbi, in_=rb)
                            # normalize in place (k side also folds 1/tau)
                            nc.vector.tensor_mul(dst[:, sl512], dst[:, sl512], rbi)

            # ---- v projection (natural orientation) ------------------------
            # v_sb[:, m, h, 0:64] = v rows m*128..+128 for head h;
            # column 64 is ones (softmax denominator trick).
            v_sb = vpool.tile([128, MQ, HPC, HD + 1], DT_BF, tag="v")
            nc.vector.memset(v_sb[:, :, :, HD:HD + 1], 1.0)
            for m in range(MQ):
                vp = ps_acc.tile([128, 256], DT_F32, tag="oacc")
                for c in range(KC_E):
                    nc.tensor.matmul(
                        vp,
                        lhsT=xv_sb[:, c, m * 128:(m + 1) * 128],
                        rhs=wv_sb[:, c, :],
                        start=(c == 0),
                        stop=False,
                    )
                nc.tensor.matmul(
                    vp,
                    lhsT=ones_row[0:1, 0:128],
                    rhs=bv_sb[0:1, :],
                    start=False,
                    stop=True,
                )
                nc.vector.tensor_copy(
                    out=v_sb[:, m, :, 0:HD],
                    in_=vp.rearrange("p (h d) -> p h d", h=HPC),
                )

            # ---- attention per head pair ------------------------------------
            heads_t = [qkpool.tile([128, S], DT_BF, tag=f"ht{p}", name=f"ht{p}") for p in range(NPAIR)]
            for p in range(NPAIR):
                for qb in range(4):  # 512-wide query blocks
                    sl_q = slice(qb * 512, (qb + 1) * 512)
                    o0 = ps_acc.tile([128, 512], DT_F32, tag="oacc")
                    o1 = ps_acc.tile([128, 512], DT_F32, tag="oacc")
                    for kc in range(MQ):
                        sc = ps_mm.tile([128, 1024], DT_F32, tag="sc")
                        nc.tensor.matmul(
                            sc[:, 0:512],
                            lhsT=kt[p][0:64, kc * 128:(kc + 1) * 128],
                            rhs=qt[p][0:64, sl_q],
                            start=True,
                            stop=True,
                        )
                        nc.tensor.matmul(
                            sc[:, 512:1024],
                            lhsT=kt[p][64:128, kc * 128:(kc + 1) * 128],
                            rhs=qt[p][64:128, sl_q],
                            start=True,
                            stop=True,
                        )
                        ex = work.tile([128, 1024], DT_BF, tag="exp")
                        nc.scalar.activation(
                            ex, sc, mybir.ActivationFunctionType.Exp
                        )
                        nc.tensor.matmul(
                            o0[0:65, :],
                            lhsT=v_sb[:, kc, 2 * p, :],
                            rhs=ex[:, 0:512],
                            start=(kc == 0),
                            stop=(kc == MQ - 1),
                        )
                        nc.tensor.matmul(
                            o1[0:65, :],
                            lhsT=v_sb[:, kc, 2 * p + 1, :],
                            rhs=ex[:, 512:1024],
                            start=(kc == 0),
                            stop=(kc == MQ - 1),
                        )
                    for hl, o in ((0, o0), (1, o1)):
                        zs = work.tile([128, 512], DT_F32, tag="zi")
                        nc.vector.tensor_copy(zs[64:65, :], o[64:65, :])
                        zb = ps_zb.tile([64, 512], DT_F32, tag="zb")
                        nc.tensor.matmul(
                            zb,
                            lhsT=ones_hi[64:65, 0:64],
                            rhs=zs[64:65, :],
                            start=True,
                            stop=True,
                        )
                        zbi = work.tile([64, 512], DT_F32, tag="ot")
                        nc.vector.reciprocal_approx_fast(out=zbi, in_=zb)
                        if hl == 0:
                            nc.vector.tensor_mul(
                                heads_t[p][0:64, sl_q], o[0:64, :], zbi
                            )
                        else:
                            t2 = work.tile([64, 512], DT_BF, tag="t2")
                            nc.vector.tensor_mul(t2, o[0:64, :], zbi)
                            nc.sync.dma_start(
                                out=heads_t[p][64:128, sl_q], in_=t2
                            )

            # ---- partial out-projection ------------------------------------
            for m in range(MQ):
                op = ps_mm.tile([128, 1024], DT_F32, tag="sc")
                for c in range(2):
                    for n2 in range(2):
                        nc.tensor.matmul(
                            op[:, n2 * 512:(n2 + 1) * 512],
                            lhsT=heads_t[c][:, m * 128:(m + 1) * 128],
                            rhs=wo_sb[:, c, n2 * 512:(n2 + 1) * 512],
                            start=(c == 0),
                            stop=(c == 1),
                        )
                ob = outs.tile([128, 1024], DT_F32, tag="ob")
                nc.vector.tensor_copy(ob, op)
                nc.sync.dma_start(out=outp[m * 128:(m + 1) * 128, :], in_=ob)

    nc.compile()
    return nc


_CACHE = {}


def _get_program():
    if "nc" not in _CACHE:
        _CACHE["nc"] = build_program()
    return _CACHE["nc"]


def make_in_maps(query, key, value, in_proj_weight, in_proj_bias,
                 out_proj_weight, out_proj_bias, tau):
    query = np.asarray(query, np.float32)
    key = np.asarray(key, np.float32)
    value = np.asarray(value, np.float32)
    W = np.asarray(in_proj_weight, np.float32)
    bias = np.asarray(in_proj_bias, np.float32)
    Wo = np.asarray(out_proj_weight, np.float32)
    tau_c = np.maximum(np.asarray(tau, np.float32).reshape(H), TAU_MIN)

    # Transposed activations per batch: (E, S) bf16
    xT = {}
    for b in range(B):
        xT["q", b] = np.ascontiguousarray(query[:, b, :].T).astype(BF16)
        xT["k", b] = np.ascontiguousarray(key[:, b, :].T).astype(BF16)
        xT["v", b] = np.ascontiguousarray(value[:, b, :].T).astype(BF16)

    selq_host = np.zeros((2, 128), np.float32)
    selq_host[0, 0:64] = 1.0
    selq_host[1, 64:128] = 1.0
    in_maps = []
    for c in range(NCORES):
        b = c // 4
        h0 = HPC * (c % 4)
        rows = slice(h0 * HD, (h0 + HPC) * HD)
        rows_k = slice(E + h0 * HD, E + (h0 + HPC) * HD)
        rows_v = slice(2 * E + h0 * HD, 2 * E + (h0 + HPC) * HD)
        # per-pair selector with 1/tau folded in for the k side
        selk = np.zeros((2, 256), np.float32)
        for mc in range(NPAIR):
            selk[0, mc * 128:mc * 128 + 64] = tau_c[h0 + 2 * mc]
            selk[1, mc * 128 + 64:(mc + 1) * 128] = tau_c[h0 + 2 * mc + 1]
        in_maps.append({
            "xq_t": xT["q", b],
            "xk_t": xT["k", b],
            "xv_t": xT["v", b],
            "wq_t": np.ascontiguousarray(W[rows, :].T).astype(BF16),
            "wk_t": np.ascontiguousarray(W[rows_k, :].T).astype(BF16),
            "wv_t": np.ascontiguousarray(W[rows_v, :].T).astype(BF16),
            "b_q": bias[rows].reshape(1, 256).astype(BF16),
            "b_k": bias[rows_k].reshape(1, 256).astype(BF16),
            "b_v": bias[rows_v].reshape(1, 256).astype(BF16),
            "wo_t": np.ascontiguousarray(Wo[:, rows].T).astype(BF16),
            "selk": selk,
            "selq": selq_host,
        })
    return in_maps


def assemble_out(results, out_proj_bias):
    bo = np.asarray(out_proj_bias, np.float32)
    out = np.zeros((S, B, E), np.float32)
    for c in range(NCORES):
        out[:, c // 4, :] += results[c]["out_p"]
    out += bo[None, None, :]
    return out


def kernel(query, key, value, in_proj_weight, in_proj_bias,
           out_proj_weight, out_proj_bias, tau):
    nc = _get_program()
    in_maps = make_in_maps(query, key, value, in_proj_weight, in_proj_bias,
                           out_proj_weight, out_proj_bias, tau)
    res = run_bass_kernel_spmd(nc, in_maps, core_ids=list(range(NCORES)))
    return assemble_out(res.results, out_proj_bias)


if __name__ == "__main__":
    import reference

    inputs = {k: np.asarray(v) for k, v in reference.setup_inputs().items()}
    out = kernel(**inputs)
    print("out shape", out.shape, out.dtype)
